# revision 11
# baseline (speedup 1.0000x reference)
"""AttentiveRNN Trainium2 kernel.

Reference semantics (per time step t over T steps, batch B):
    h_t = relu(x_t @ W_in.T + b_in)
    c_t = relu([c_{t-1}; h_t] @ W_ctx.T + b_ctx)
    key_{t+1} = c_t @ W_key.T + b_key     (key_0 from c0)
    q_t = c_t @ W_q.T + b_q
    scores_s = key_s . q_t   for s <= t+1, softmax over s
    w_t = sum_s attn_s * ctx_s ;  actions_t = w_t @ W_act.T + b_act

Device strategy (data-parallel over batch, 64 per core, feature-major):
  Phase 1: all h_t via one big matmul (x.T pre-transposed on host).
  Phase 2: sequential scan for c_t (256 steps, PE matmul + ACT relu).
  Phase 3: keys/queries for all contexts via one big matmul.
  Phase 4: per batch element: S^T = K Q^T on PE, exp on ACT, causal mask via
           affine_select on DVE, then actions_unnorm/denominator via PE
           matmuls against [C @ W_act.T | 1].
  Host: softmax normalization, the s=T edge term (only visible at t=T-1),
        and + b_act. These are O(T*B*A) vectorized numpy.
"""

import sys

sys.path.insert(0, "/opt/trn_rl_repo")

import numpy as np

import concourse.bacc as bacc
import concourse.bass as bass
import concourse.tile as tile
from concourse import bass_utils, mybir

T, B, D, H, K, A = 256, 512, 128, 50, 5, 4
N_CORES = 8
BC = B // N_CORES  # 64 batch elements per core
S = T + 1  # context count (c_{-1}=c0 .. c_{T-1})
NX = T * BC  # 16384 columns of x per core
F32 = mybir.dt.float32
AF = mybir.ActivationFunctionType

_CACHE = {}


def _build_nc():
    nc = bacc.Bacc("TRN2", target_bir_lowering=False, debug=False)

    xT = nc.dram_tensor("xT", [D, NX], F32, kind="ExternalInput")
    ones_row = nc.dram_tensor("ones_row", [14, S, BC], F32, kind="ExternalInput")
    w_p1 = nc.dram_tensor("w_p1", [D, H], F32, kind="ExternalInput")
    b_in_c = nc.dram_tensor("b_in_c", [H, 1], F32, kind="ExternalInput")
    w_ch = nc.dram_tensor("w_ch", [64 + H, H], F32, kind="ExternalInput")
    b_ctx_c = nc.dram_tensor("b_ctx_c", [H, 1], F32, kind="ExternalInput")
    m_hat = nc.dram_tensor("m_hat", [H + 1, H + 1], F32, kind="ExternalInput")
    w_ae = nc.dram_tensor("w_ae", [H + 1, K], F32, kind="ExternalInput")
    c0_t = nc.dram_tensor("c0_t", [H, BC], F32, kind="ExternalInput")

    acts_raw = nc.dram_tensor("acts_raw", [128, 2, BC, 5], F32, kind="ExternalOutput")
    c_last = nc.dram_tensor("c_last", [H, 1, BC], F32, kind="ExternalOutput")

    with tile.TileContext(nc) as tc:
        with (
            tc.tile_pool(name="persist", bufs=1) as persist,
            tc.tile_pool(name="xchunks", bufs=2) as xpool,
            tc.tile_pool(name="epool", bufs=4) as epool,
            tc.tile_pool(name="caepool", bufs=2) as caepool,
        ):
            # CAT rows: 0-49 context c, 50 ones, 64-113 h (engine APs must
            # start at partition 0/32/64/96).  Column block s holds c_{s-1}
            # (block 0 = c0) and h_s.
            CAT = persist.tile([64 + H, S, BC], F32)
            ACTS = persist.tile([128, 2, BC, K], F32)

            wp1_sb = persist.tile([D, H], F32, tag="wp1")
            nc.sync.dma_start(wp1_sb, w_p1[:])
            bin_sb = persist.tile([H, 1], F32, tag="bin")
            nc.sync.dma_start(bin_sb, b_in_c[:])
            wch_sb = persist.tile([64 + H, H], F32, tag="wch")
            nc.sync.dma_start(wch_sb, w_ch[:])
            bctx_sb = persist.tile([H, 1], F32, tag="bctx")
            nc.sync.dma_start(bctx_sb, b_ctx_c[:])
            mh_sb = persist.tile([H + 1, H + 1], F32, tag="mh")
            nc.sync.dma_start(mh_sb, m_hat[:])
            wae_sb = persist.tile([H + 1, K], F32, tag="wae")
            nc.sync.dma_start(wae_sb, w_ae[:])

            # row 50 = ones; rows 51-63 = zeros (padding under the h block)
            nc.sync.dma_start(CAT[H:64, :, :], ones_row[:])
            nc.sync.dma_start(CAT[0:H, 0, :], c0_t[:])

            with (
                tc.tile_pool(name="psA", bufs=2, space=bass.MemorySpace.PSUM) as psA,
                tc.tile_pool(name="psB", bufs=2, space=bass.MemorySpace.PSUM) as psB,
            ):
                # ---- Phase 1: h for all t ----
                NCHUNK = 8
                CW = NX // NCHUNK  # 4096 columns (= 64 t-blocks)
                TB = CW // BC  # 64 t per chunk
                for kc in range(NCHUNK):
                    xt = xpool.tile([D, CW], F32, tag="xt")
                    nc.sync.dma_start(xt[:], xT[:, kc * CW : (kc + 1) * CW])
                    for j in range(CW // 512):
                        ps = psA.tile([H, 512], F32, tag="mm1")
                        nc.tensor.matmul(ps, wp1_sb, xt[:, j * 512 : (j + 1) * 512])
                        t0 = kc * TB + j * (512 // BC)
                        nc.scalar.activation(
                            CAT[64 : 64 + H, t0 : t0 + 512 // BC, :],
                            ps,
                            AF.Relu,
                            bias=bin_sb,
                        )

                # ---- Phase 2: sequential context scan ----
                for t in range(T):
                    ps = psB.tile([H, BC], F32, tag="chain")
                    nc.tensor.matmul(ps, wch_sb, CAT[:, t, :])
                    nc.scalar.activation(
                        CAT[0:H, t + 1, :], ps, AF.Relu, bias=bctx_sb
                    )

            # ---- Phase 4: attention per batch element ----
            with (
                tc.tile_pool(name="gpool", bufs=3) as gpool,
                tc.tile_pool(name="psS", bufs=2, space=bass.MemorySpace.PSUM) as psS,
                tc.tile_pool(name="psG", bufs=2, space=bass.MemorySpace.PSUM) as psG,
                tc.tile_pool(name="psC", bufs=2, space=bass.MemorySpace.PSUM) as psC,
            ):
                for g in range(BC // 8):
                    caps = psC.tile([128, 2, 8, K], F32, tag="ca")
                    acps = psC.tile([128, 2, 8, K], F32, tag="ac")
                    cae = caepool.tile([128, 2, 8, K], F32, tag="cae")
                    for bi in range(8):
                        b = g * 8 + bi
                        for sc in range(2):
                            # CA_ext[s,:] = [C[s] @ W_act.T | 1] for this b
                            nc.tensor.matmul(
                                caps[:, sc, bi, :],
                                CAT[0 : H + 1, sc * 128 : (sc + 1) * 128, b],
                                wae_sb,
                            )
                    nc.vector.tensor_copy(cae, caps)
                    for bi in range(8):
                        b = g * 8 + bi
                        # G = M_hat @ [C;1] for this b: S[s,t] = chat_s . G[:,t]
                        gps = psG.tile([H + 1, S], F32, tag="g")
                        nc.tensor.matmul(gps, mh_sb, CAT[0 : H + 1, :, b])
                        gsb = gpool.tile([H + 1, S], F32, tag="gsb")
                        nc.vector.tensor_copy(gsb, gps)
                        e_tiles = []
                        for sc in range(2):
                            stp = psS.tile([128, T], F32, tag="st")
                            nc.tensor.matmul(
                                stp,
                                CAT[0 : H + 1, sc * 128 : (sc + 1) * 128, b],
                                gsb[:, 1:S],
                            )
                            e = epool.tile([128, T], F32, tag="e")
                            nc.scalar.activation(e, stp, AF.Exp)
                            # keep only s <= t+1:  iota = t - p + (1 - sc*128) >= 0
                            nc.gpsimd.affine_select(
                                e,
                                e,
                                pattern=[[1, T]],
                                compare_op=mybir.AluOpType.is_ge,
                                fill=0.0,
                                base=1 - sc * 128,
                                channel_multiplier=-1,
                            )
                            e_tiles.append(e)
                        for tcn in range(2):
                            for sc in range(2):
                                nc.tensor.matmul(
                                    acps[:, tcn, bi, :],
                                    e_tiles[sc][:, tcn * 128 : (tcn + 1) * 128],
                                    cae[:, sc, bi, :],
                                    start=(sc == 0),
                                    stop=(sc == 1),
                                )
                    nc.vector.tensor_copy(ACTS[:, :, g * 8 : (g + 1) * 8, :], acps)

            nc.sync.dma_start(acts_raw[:], ACTS[:])
            nc.sync.dma_start(c_last[:], CAT[0:H, S - 1 : S, :])

    nc.compile()
    return nc


def _get_nc():
    if "nc" not in _CACHE:
        _CACHE["nc"] = _build_nc()
    return _CACHE["nc"]


def _prep_inputs(x, W_in, b_in, W_ctx, b_ctx, W_key, b_key, W_q, b_q,
                 first_context, W_act, b_act):
    x = np.asarray(x, np.float32)
    shared = {
        "ones_row": np.concatenate(
            [np.ones((1, S, BC), np.float32), np.zeros((13, S, BC), np.float32)]
        ),
        "w_p1": np.ascontiguousarray(np.asarray(W_in, np.float32).T),
        "b_in_c": np.asarray(b_in, np.float32).reshape(H, 1),
        "b_ctx_c": np.asarray(b_ctx, np.float32).reshape(H, 1),
        "c0_t": np.ascontiguousarray(
            np.broadcast_to(np.asarray(first_context, np.float32)[:, None], (H, BC))
        ),
    }
    w_ch = np.zeros((64 + H, H), np.float32)
    w_ch[0:H] = np.asarray(W_ctx, np.float32)[:, 0:H].T
    w_ch[64:] = np.asarray(W_ctx, np.float32)[:, H:].T
    shared["w_ch"] = w_ch
    # m_hat: S[s,t] = chat_s^T Mh chat_t with chat=[c;1]; device computes
    # G = Mh @ chat_t via matmul(lhsT=m_hat) so pass Mh^T ( = Mh' with
    # Mh'[i,j] = Mh[j,i]).
    Wk = np.asarray(W_key, np.float64)
    Wq = np.asarray(W_q, np.float64)
    bk = np.asarray(b_key, np.float64)
    bq = np.asarray(b_q, np.float64)
    mh = np.zeros((H + 1, H + 1), np.float64)
    mh[0:H, 0:H] = Wk.T @ Wq
    mh[0:H, H] = Wk.T @ bq
    mh[H, 0:H] = bk @ Wq
    mh[H, H] = bk @ bq
    shared["m_hat"] = np.ascontiguousarray(mh.T).astype(np.float32)
    w_ae = np.zeros((H + 1, K), np.float32)
    w_ae[0:H, 0:A] = np.asarray(W_act, np.float32).T
    w_ae[H, A] = 1.0
    shared["w_ae"] = w_ae

    in_maps = []
    for c in range(N_CORES):
        xc = x[:, c * BC : (c + 1) * BC, :]  # [T, BC, D]
        xTc = np.ascontiguousarray(xc.transpose(2, 0, 1)).reshape(D, NX)
        in_maps.append({"xT": xTc, **shared})
    return in_maps


def _postprocess(results, W_key, b_key, W_q, b_q, W_act, b_act):
    W_key = np.asarray(W_key, np.float64)
    W_q = np.asarray(W_q, np.float64)
    W_act = np.asarray(W_act, np.float64)
    b_key = np.asarray(b_key, np.float64)
    b_q = np.asarray(b_q, np.float64)
    b_act = np.asarray(b_act, np.float32)
    out = np.empty((T, B, A), np.float32)
    for c in range(N_CORES):
        raw = np.asarray(results[c]["acts_raw"], np.float64)  # [128, 2, BC, 5]
        cl = np.asarray(results[c]["c_last"], np.float64).reshape(H, BC)
        tmp = raw.transpose(1, 0, 2, 3).reshape(T, BC, K)
        num = tmp[..., 0:A]
        den = tmp[..., A]
        # s = T edge term: only t = T-1 attends to key_T (from c_{T-1})
        key_T = W_key @ cl + b_key[:, None]  # [K, BC]
        q_last = W_q @ cl + b_q[:, None]
        e_edge = np.exp((key_T * q_last).sum(0))  # [BC]
        ca_edge = cl.T @ W_act.T  # [BC, A]
        num[T - 1] += e_edge[:, None] * ca_edge
        den[T - 1] += e_edge
        out[:, c * BC : (c + 1) * BC, :] = (num / den[..., None]).astype(
            np.float32
        ) + b_act
    return out


def _get_runner():
    """Jitted SPMD executor, built once and cached (run_bass_via_pjrt builds a
    fresh closure per call, which re-traces every time)."""
    if "runner" in _CACHE:
        return _CACHE["runner"]
    import jax
    from jax.experimental.shard_map import shard_map
    from jax.sharding import Mesh, PartitionSpec

    from concourse import bass2jax, mybir as mb

    nc = _get_nc()
    bass2jax.install_neuronx_cc_hook()
    assert nc.dbg_addr is None
    partition_name = nc.partition_id_tensor.name if nc.partition_id_tensor else None

    in_names, out_names, out_avals, zero_outs = [], [], [], []
    for alloc in nc.m.functions[0].allocations:
        if not isinstance(alloc, mb.MemoryLocationSet):
            continue
        name = alloc.memorylocations[0].name
        if alloc.kind == "ExternalInput":
            in_names.append(name)
        elif alloc.kind == "ExternalOutput":
            shape = tuple(alloc.tensor_shape)
            dtype = mb.dt.np(alloc.dtype)
            out_names.append(name)
            out_avals.append(jax.core.ShapedArray(shape, dtype))
            zero_outs.append(np.zeros(shape, dtype))
    if partition_name is not None:
        in_names = [n for n in in_names if n != partition_name]
    n_params = len(in_names)
    all_names = in_names + out_names
    if partition_name is not None:
        all_names = all_names + [partition_name]
    donate = tuple(range(n_params, n_params + len(out_names)))

    def _body(*args):
        operands = list(args)
        if partition_name is not None:
            operands.append(bass2jax.partition_id_tensor())
        outs = bass2jax._bass_exec_p.bind(
            *operands,
            out_avals=tuple(out_avals),
            in_names=tuple(all_names),
            out_names=tuple(out_names),
            lowering_input_output_aliases=(),
            sim_require_finite=True,
            sim_require_nnan=True,
            nc=nc,
        )
        return tuple(outs)

    devices = jax.devices()[:N_CORES]
    mesh = Mesh(np.asarray(devices), ("core",))
    specs = (PartitionSpec("core"),) * (n_params + len(out_names))
    sharded = jax.jit(
        shard_map(
            _body,
            mesh=mesh,
            in_specs=specs,
            out_specs=(PartitionSpec("core"),) * len(out_names),
            check_rep=False,
        ),
        donate_argnums=donate,
        keep_unused=True,
    )

    def run(in_maps):
        concat_in = [
            np.concatenate([in_maps[c][n] for c in range(N_CORES)], axis=0)
            for n in in_names
        ]
        concat_zero = [
            np.zeros((N_CORES * z.shape[0], *z.shape[1:]), z.dtype) for z in zero_outs
        ]
        out_arrs = sharded(*concat_in, *concat_zero)
        return [
            {
                n: np.asarray(out_arrs[i]).reshape(N_CORES, *out_avals[i].shape)[c]
                for i, n in enumerate(out_names)
            }
            for c in range(N_CORES)
        ]

    run.sharded = sharded
    run.in_names = in_names
    run.out_names = out_names
    run.out_avals = out_avals
    run.zero_outs = zero_outs
    _CACHE["runner"] = run
    return run


def kernel(**inputs):
    run = _get_runner()
    in_maps = _prep_inputs(**inputs)
    results = run(in_maps)
    return _postprocess(
        results,
        inputs["W_key"],
        inputs["b_key"],
        inputs["W_q"],
        inputs["b_q"],
        inputs["W_act"],
        inputs["b_act"],
    )
